# revision 16
# baseline (speedup 1.0000x reference)
"""Trainium2 Bass kernel for masked multi-head attention with a rope-like
positional transform (nn_Attention_43937515438607).

Math per reference:
    qkv = x @ W_qkv.T + b_qkv                     (B,T,3C)
    q,k,v = split(qkv);  heads of D=64
    q = (q*pe0 + rot(q)*pe1) * pe2
    k = (k*pe0 + rot(k)*pe1) / pe2
    S = q k^T / sqrt(2D);  S[mask] = -inf;  alpha = softmax(S)
    out = alpha @ v  ->  (B,T,C)

Device strategy (8 cores, 2 batches per core):
  - projection as natural-layout matmuls (fp32r), bias via K=1 ones-row matmul
  - rope applied on VectorE in natural layout (bf16 out)
  - Q,K transposed on TensorE to [d, t] layout for the S^T matmul
  - S^T = K^T.T @ Q^T per (batch, head-pair, kv-tile) with contraction d=64
    (two heads run concurrently in disjoint row-groups of the PE array)
  - softmax without max-subtraction: exp on ScalarE straight out of PSUM,
    mask applied as a bf16 multiply with a host-pretransposed (1-mask),
    denominator from a ones-column appended to V in the AV matmul
  - O^T = V_ext.T @ alpha^T accumulated over kv tiles (M=65: 64 d + denom)
  - final transpose back to [t, c] on TensorE, divide by denom on VectorE
"""

import sys

try:
    import concourse  # noqa: F401
except ImportError:  # pragma: no cover
    sys.path.insert(0, "/opt/trn_rl_repo")

import numpy as np
import ml_dtypes

from concourse import bass, mybir, tile, bacc
from concourse.bass_utils import run_bass_kernel_spmd
from concourse.masks import make_identity

# problem constants (hardcoded per harness contract)
B, T, C = 16, 1024, 512
NH = 8
D = C // NH
TP = float((2.0 * D) ** 0.5)
N_CORES = 8
BPC = B // N_CORES            # batches per core = 2
TOK = BPC * T                 # tokens per core  = 2048
NTT = TOK // 128              # token tiles per core = 16
NTB = T // 128                # token tiles per batch = 8
NHP = NH // 2                 # head pairs = 4
QC = 512                      # q chunk (PSUM bank) per attention unit
NQC = T // QC                 # q chunks per batch = 2

F32 = mybir.dt.float32
F32R = mybir.dt.float32r
BF16 = mybir.dt.bfloat16


def _r(ap):
    """bitcast an fp32 AP to fp32r for full-rate matmul"""
    return ap.bitcast(F32R)


def build_nc():
    nc = bacc.Bacc("TRN2", target_bir_lowering=False, debug=False)

    # ---- DRAM I/O ----
    xT_d = nc.dram_tensor("xT", [C, TOK], F32R, kind="ExternalInput")
    wT_d = nc.dram_tensor("wT", [C, 3 * C], F32R, kind="ExternalInput")
    brow_d = nc.dram_tensor("brow", [1, 3 * C], F32R, kind="ExternalInput")
    ones_d = nc.dram_tensor("ones_row", [1, TOK], F32R, kind="ExternalInput")
    peAq_d = nc.dram_tensor("peAq", [T, D], F32, kind="ExternalInput")
    peBq_d = nc.dram_tensor("peBq", [T, D], F32, kind="ExternalInput")
    peAk_d = nc.dram_tensor("peAk", [T, D], F32, kind="ExternalInput")
    peBk_d = nc.dram_tensor("peBk", [T, D], F32, kind="ExternalInput")
    nmT_d = nc.dram_tensor("nmT", [BPC, T, T], BF16, kind="ExternalInput")
    y_d = nc.dram_tensor("y", [TOK, C], F32, kind="ExternalOutput")

    with tile.TileContext(nc) as tc:
        import contextlib
        ctx = contextlib.ExitStack()
        with ctx:
            persist = ctx.enter_context(tc.tile_pool(name="persist", bufs=1))

            # persistent through attention/final
            V_sb = persist.tile([128, NTT, NH * 66 + 32], BF16)     # 17.5KB/part
            QT = [persist.tile([128, NHP, T], F32R, tag=f"QT{b}", name=f"QT{b}")
                  for b in range(BPC)]
            KT = [persist.tile([128, NHP, T], F32R, tag=f"KT{b}", name=f"KT{b}")
                  for b in range(BPC)]
            id_bf = persist.tile([128, 128], BF16)
            id_tmp = persist.tile([128, 128], F32)
            id_f32 = persist.tile([128, 128], F32R)

            make_identity(nc, id_bf[:])
            make_identity(nc, id_tmp[:])
            nc.vector.tensor_copy(id_f32[:], id_tmp[:])
            # V_ext: zero padding cols, ones in the denominator column
            nc.gpsimd.memset(V_sb[:], 0.0)
            nc.vector.memset(V_sb[:, :, 64::66], 1.0)

            # ---------- phase 1: projection + rope + transposes ----------
            with tc.tile_pool(name="projin", bufs=1) as projin, \
                 tc.tile_pool(name="xin", bufs=3) as xin_pool, \
                 tc.tile_pool(name="qkn", bufs=3) as qkn_pool, \
                 tc.tile_pool(name="proj_ps", bufs=4, space="PSUM") as proj_ps, \
                 tc.tile_pool(name="tp_ps", bufs=4, space="PSUM") as tp_ps, \
                 tc.tile_pool(name="ropet", bufs=3) as ropet:

                wT = projin.tile([128, 4, 3 * C], F32R)
                ones_sb = projin.tile([1, TOK], F32R)
                brow = projin.tile([1, 3 * C], F32R)
                peA = [projin.tile([128, NTB, D], F32, tag=f"peA{i}", name=f"peA{i}")
                       for i in range(2)]
                peB = [projin.tile([128, NTB, D], F32, tag=f"peB{i}", name=f"peB{i}")
                       for i in range(2)]

                nc.sync.dma_start(wT[:], wT_d.rearrange("(ko p) f -> p ko f", p=128))
                nc.sync.dma_start(ones_sb[:], ones_d[:])
                nc.sync.dma_start(brow[:], brow_d[:])
                for i, d in enumerate([peAq_d, peAk_d]):
                    nc.sync.dma_start(peA[i][:], d.rearrange("(tb p) d -> p tb d", p=128))
                for i, d in enumerate([peBq_d, peBk_d]):
                    nc.sync.dma_start(peB[i][:], d.rearrange("(tb p) d -> p tb d", p=128))

                for tt in range(NTT):
                    b, ttb = tt // NTB, tt % NTB
                    xt = xin_pool.tile([128, 4, 128], F32R, tag="xt")
                    nc.sync.dma_start(
                        xt[:],
                        xT_d[:, tt * 128:(tt + 1) * 128].rearrange(
                            "(ko p) t -> p ko t", p=128))
                    for fc in range(3):
                        ps = proj_ps.tile([128, 512], F32, tag="proj")
                        for kk in range(4):
                            nc.tensor.matmul(
                                ps[:], xt[:, kk, :],
                                wT[:, kk, fc * 512:(fc + 1) * 512],
                                start=(kk == 0), stop=False)
                        nc.tensor.matmul(
                            ps[:],
                            ones_sb[:, tt * 128:(tt + 1) * 128],
                            brow[:, fc * 512:(fc + 1) * 512],
                            start=False, stop=True)

                        if fc < 2:  # Q or K: rope -> fp32, then transpose
                            A = peA[fc][:, ttb, :]
                            Bp = peB[fc][:, ttb, :]
                            ps3 = ps[:].rearrange("p (h d) -> p h d", h=NH)
                            qk = qkn_pool.tile([128, C], F32R, tag=f"qk{fc}",
                                               name=f"qk{fc}")
                            t1 = ropet.tile([128, NH, D], F32, tag="t1")
                            nc.vector.tensor_tensor(
                                t1[:], ps3,
                                A[:, None, :].to_broadcast([128, NH, D]),
                                mybir.AluOpType.mult)
                            t2 = ropet.tile([128, NH, D], F32, tag="t2")
                            ps4 = ps[:].rearrange(
                                "p (h x two) -> p h x two", h=NH, two=2)
                            ps4_sw = ps4[:, :, :, ::-1]
                            nc.vector.tensor_tensor(
                                t2[:].rearrange("p h (x two) -> p h x two", two=2),
                                ps4_sw,
                                Bp[:, None, :].rearrange(
                                    "p o (x two) -> p o x two", two=2
                                ).to_broadcast([128, NH, D // 2, 2]),
                                mybir.AluOpType.mult)
                            nc.vector.tensor_tensor(
                                qk[:].rearrange("p (h d) -> p h d", h=NH),
                                t1[:], t2[:], mybir.AluOpType.add)
                            # transpose all 4 head-pairs of this token tile
                            tp = tp_ps.tile([128, 512], F32R, tag="tp")
                            for hp in range(NHP):
                                nc.tensor.matmul(
                                    tp[:, hp * 128:(hp + 1) * 128],
                                    qk[:, hp * 128:(hp + 1) * 128],
                                    id_f32[:],
                                    is_transpose=True)
                            dstt = (QT if fc == 0 else KT)[b]
                            nc.vector.tensor_copy(
                                dstt[:, :, ttb * 128:(ttb + 1) * 128],
                                tp[:].rearrange("p (hp t) -> p hp t", hp=NHP))
                        else:  # V: copy into V_ext layout (skip ones cols)
                            nc.vector.tensor_copy(
                                V_sb[:, tt, :528].rearrange(
                                    "p (h e) -> p h e", h=NH)[:, :, :D],
                                ps[:].rearrange("p (h d) -> p h d", h=NH))

            # ---------- phase 3: attention ----------
            persist2 = ctx.enter_context(tc.tile_pool(name="persist2", bufs=1))
            OT = [persist2.tile([96, NH, T], F32R, tag=f"OT{b}", name=f"OT{b}")
                  for b in range(BPC)]
            mT = [persist2.tile([128, NTB, T], BF16, tag=f"mT{b}", name=f"mT{b}")
                  for b in range(BPC)]
            for b in range(BPC):
                nc.sync.dma_start(
                    mT[b][:], nmT_d[b].rearrange("(kt p) q -> p kt q", p=128))

            with tc.tile_pool(name="s_ps", bufs=2, space="PSUM") as s_ps, \
                 tc.tile_pool(name="o_ps", bufs=2, space="PSUM") as o_ps, \
                 tc.tile_pool(name="alpha", bufs=3) as alpha_pool:

                for b in range(BPC):
                    for hp in range(NHP):
                        hA, hB = 2 * hp, 2 * hp + 1
                        for qc in range(NQC):
                            oA = o_ps.tile([96, QC], F32, tag="oA")
                            oB = o_ps.tile([96, QC], F32, tag="oB")
                            for kt in range(NTB):
                                sp = s_ps.tile([128, 2 * QC], F32, tag="s")
                                # S^T halves: head A cols 0:512, head B 512:1024
                                nc.tensor.matmul(
                                    sp[:, 0:QC],
                                    KT[b][0:64, hp, kt * 128:(kt + 1) * 128],
                                    QT[b][0:64, hp, qc * QC:(qc + 1) * QC],
                                    start=True, stop=True)
                                nc.tensor.matmul(
                                    sp[:, QC:2 * QC],
                                    KT[b][64:128, hp, kt * 128:(kt + 1) * 128],
                                    QT[b][64:128, hp, qc * QC:(qc + 1) * QC],
                                    start=True, stop=True)
                                al = alpha_pool.tile([128, 2 * QC], BF16, tag="al")
                                nc.scalar.activation(
                                    al[:], sp[:],
                                    mybir.ActivationFunctionType.Exp,
                                    scale=1.0 / TP)
                                nc.vector.tensor_tensor(
                                    al[:].rearrange("p (h q) -> p h q", h=2),
                                    al[:].rearrange("p (h q) -> p h q", h=2),
                                    mT[b][:, kt, qc * QC:(qc + 1) * QC][:, None, :]
                                    .to_broadcast([128, 2, QC]),
                                    mybir.AluOpType.mult)
                                vbase = b * NTB + kt
                                nc.tensor.matmul(
                                    oA[:],
                                    V_sb[:, vbase, hA * 66:hA * 66 + 96],
                                    al[:, 0:QC],
                                    start=(kt == 0), stop=(kt == NTB - 1))
                                nc.tensor.matmul(
                                    oB[:],
                                    V_sb[:, vbase, hB * 66:hB * 66 + 96],
                                    al[:, QC:2 * QC],
                                    start=(kt == 0), stop=(kt == NTB - 1))
                            nc.vector.tensor_copy(
                                OT[b][:, hA, qc * QC:(qc + 1) * QC], oA[:])
                            nc.vector.tensor_copy(
                                OT[b][:, hB, qc * QC:(qc + 1) * QC], oB[:])

            # ---------- phase 4: final transpose + normalize + store ----------
            with tc.tile_pool(name="fin_ps", bufs=4, space="PSUM") as fin_ps, \
                 tc.tile_pool(name="fin_sb", bufs=3) as fin_sb:
                for b in range(BPC):
                    for qt in range(NTB):
                        fins = []
                        for half in range(2):
                            fp = fin_ps.tile([128, 4 * 96], F32R, tag=f"fin{half}",
                                             name=f"fin{half}")
                            for hh in range(4):
                                h = half * 4 + hh
                                nc.tensor.matmul(
                                    fp[:, hh * 96:(hh + 1) * 96],
                                    OT[b][:, h, qt * 128:(qt + 1) * 128],
                                    id_f32[0:96, 0:96],
                                    is_transpose=True)
                            fins.append(fp)
                        out_sb = fin_sb.tile([128, C], F32, tag="out")
                        for half in range(2):
                            fp = fins[half]
                            rc = fin_sb.tile([128, 4], F32, tag=f"rc{half}",
                                             name=f"rc{half}")
                            nc.vector.reciprocal(rc[:], fp[:, 64::96])
                            nc.vector.tensor_tensor(
                                out_sb[:, half * 256:(half + 1) * 256].rearrange(
                                    "p (h d) -> p h d", h=4),
                                fp[:].rearrange("p (h e) -> p h e", e=96)[:, :, :D],
                                rc[:][:, :, None].to_broadcast([128, 4, D]),
                                mybir.AluOpType.mult)
                        nc.sync.dma_start(
                            y_d[(b * NTB + qt) * 128:(b * NTB + qt + 1) * 128, :],
                            out_sb[:])

    nc.compile()
    return nc


_NC_CACHE = None


def _get_nc():
    global _NC_CACHE
    if _NC_CACHE is None:
        _NC_CACHE = build_nc()
    return _NC_CACHE


def prep_inputs(x, pe0, pe1, pe2, mask, W_qkv, b_qkv):
    """Host-side layout prep + per-core sharding. Returns list of in_maps."""
    x = np.asarray(x, dtype=np.float32)
    pe0 = np.asarray(pe0, dtype=np.float32).reshape(T, D)
    pe1 = np.asarray(pe1, dtype=np.float32).reshape(T, D)
    pe2 = np.asarray(pe2, dtype=np.float32).reshape(T, D)
    mask = np.asarray(mask).reshape(B, T, T)
    W_qkv = np.asarray(W_qkv, dtype=np.float32)
    b_qkv = np.asarray(b_qkv, dtype=np.float32)

    wT = np.ascontiguousarray(W_qkv.T)                      # [C, 3C]
    brow = np.ascontiguousarray(b_qkv[None, :])             # [1, 3C]
    ones_row = np.ones((1, TOK), dtype=np.float32)

    # rope tables: q' = q*A + swap(q)*B' ; A=pe0*pe2, B=pe1*pe2 (sign-folded)
    Aq = pe0 * pe2
    Bq = pe1 * pe2
    Ak = pe0 / pe2
    Bk = pe1 / pe2
    sign = np.ones((T, D), dtype=np.float32)
    sign[:, 0::2] = -1.0
    peBq = np.ascontiguousarray(Bq * sign)
    peBk = np.ascontiguousarray(Bk * sign)

    notmask = (~mask).astype(ml_dtypes.bfloat16)            # [B,T,T] {0,1}
    in_maps = []
    for c in range(N_CORES):
        bs = slice(c * BPC, (c + 1) * BPC)
        xc = np.ascontiguousarray(
            x[bs].reshape(TOK, C).T)                         # [C, TOK]
        nmT = np.ascontiguousarray(
            notmask[bs].transpose(0, 2, 1))                  # [BPC, T(kv), T(q)]
        in_maps.append(dict(
            xT=xc, wT=wT, brow=brow, ones_row=ones_row,
            peAq=np.ascontiguousarray(Aq), peBq=peBq,
            peAk=np.ascontiguousarray(Ak), peBk=peBk,
            nmT=nmT,
        ))
    return in_maps


def assemble_output(results):
    out = np.empty((B, T, C), dtype=np.float32)
    for c in range(N_CORES):
        out[c * BPC:(c + 1) * BPC] = results[c]["y"].reshape(BPC, T, C)
    return out


def kernel(x, pe0, pe1, pe2, mask, W_qkv, b_qkv):
    nc = _get_nc()
    in_maps = prep_inputs(x, pe0, pe1, pe2, mask, W_qkv, b_qkv)
    res = run_bass_kernel_spmd(nc, in_maps, core_ids=list(range(N_CORES)))
    return assemble_output(res.results)


# revision 17
# speedup vs baseline: 188.5662x; 188.5662x over previous
"""Trainium2 Bass kernel for masked multi-head attention with a rope-like
positional transform (nn_Attention_43937515438607).

Math per reference:
    qkv = x @ W_qkv.T + b_qkv                     (B,T,3C)
    q,k,v = split(qkv);  heads of D=64
    q = (q*pe0 + rot(q)*pe1) * pe2
    k = (k*pe0 + rot(k)*pe1) / pe2
    S = q k^T / sqrt(2D);  S[mask] = -inf;  alpha = softmax(S)
    out = alpha @ v  ->  (B,T,C)

Device strategy (8 cores, 2 batches per core):
  - projection as natural-layout matmuls (fp32r), bias via K=1 ones-row matmul
  - rope applied on VectorE in natural layout (bf16 out)
  - Q,K transposed on TensorE to [d, t] layout for the S^T matmul
  - S^T = K^T.T @ Q^T per (batch, head-pair, kv-tile) with contraction d=64
    (two heads run concurrently in disjoint row-groups of the PE array)
  - softmax without max-subtraction: exp on ScalarE straight out of PSUM,
    mask applied as a bf16 multiply with a host-pretransposed (1-mask),
    denominator from a ones-column appended to V in the AV matmul
  - O^T = V_ext.T @ alpha^T accumulated over kv tiles (M=65: 64 d + denom)
  - final transpose back to [t, c] on TensorE, divide by denom on VectorE
"""

import sys

try:
    import concourse  # noqa: F401
except ImportError:  # pragma: no cover
    sys.path.insert(0, "/opt/trn_rl_repo")

import numpy as np
import ml_dtypes

from concourse import bass, mybir, tile, bacc
from concourse.bass_utils import run_bass_kernel_spmd
from concourse.masks import make_identity

# problem constants (hardcoded per harness contract)
B, T, C = 16, 1024, 512
NH = 8
D = C // NH
TP = float((2.0 * D) ** 0.5)
N_CORES = 8
BPC = B // N_CORES            # batches per core = 2
TOK = BPC * T                 # tokens per core  = 2048
NTT = TOK // 128              # token tiles per core = 16
NTB = T // 128                # token tiles per batch = 8
NHP = NH // 2                 # head pairs = 4
QC = 512                      # q chunk (PSUM bank) per attention unit
NQC = T // QC                 # q chunks per batch = 2

F32 = mybir.dt.float32
F32R = mybir.dt.float32r
BF16 = mybir.dt.bfloat16


def _r(ap):
    """bitcast an fp32 AP to fp32r for full-rate matmul"""
    return ap.bitcast(F32R)


def build_nc(niter=1):
    nc = bacc.Bacc("TRN2", target_bir_lowering=False, debug=False)

    # ---- DRAM I/O ----
    xT_d = nc.dram_tensor("xT", [C, TOK], F32R, kind="ExternalInput")
    wT_d = nc.dram_tensor("wT", [C, 3 * C], F32R, kind="ExternalInput")
    brow_d = nc.dram_tensor("brow", [1, 3 * C], F32R, kind="ExternalInput")
    ones_d = nc.dram_tensor("ones_row", [1, TOK], F32R, kind="ExternalInput")
    peAq_d = nc.dram_tensor("peAq", [T, D], F32, kind="ExternalInput")
    peBq_d = nc.dram_tensor("peBq", [T, D], F32, kind="ExternalInput")
    peAk_d = nc.dram_tensor("peAk", [T, D], F32, kind="ExternalInput")
    peBk_d = nc.dram_tensor("peBk", [T, D], F32, kind="ExternalInput")
    nmT_d = nc.dram_tensor("nmT", [BPC, T, T], BF16, kind="ExternalInput")
    y_d = nc.dram_tensor("y", [TOK, C], F32, kind="ExternalOutput")

    with tile.TileContext(nc) as tc:
        import contextlib
        loop_cm = tc.For_i(0, niter, 1) if niter > 1 else contextlib.nullcontext()
        ctx = contextlib.ExitStack()
        with loop_cm, ctx:
            persist = ctx.enter_context(tc.tile_pool(name="persist", bufs=1))

            # persistent through attention/final
            V_sb = persist.tile([128, NTT, NH * 66 + 32], BF16)     # 17.5KB/part
            QT = [persist.tile([128, NHP, T], F32R, tag=f"QT{b}", name=f"QT{b}")
                  for b in range(BPC)]
            KT = [persist.tile([128, NHP, T], F32R, tag=f"KT{b}", name=f"KT{b}")
                  for b in range(BPC)]
            id_bf = persist.tile([128, 128], BF16)
            id_tmp = persist.tile([128, 128], F32)
            id_f32 = persist.tile([128, 128], F32R)

            make_identity(nc, id_bf[:])
            make_identity(nc, id_tmp[:])
            nc.vector.tensor_copy(id_f32[:], id_tmp[:])
            # V_ext: zero padding cols, ones in the denominator column
            nc.gpsimd.memset(V_sb[:], 0.0)
            nc.vector.memset(V_sb[:, :, 64::66], 1.0)

            # ---------- phase 1: projection + rope + transposes ----------
            with tc.tile_pool(name="projin", bufs=1) as projin, \
                 tc.tile_pool(name="xin", bufs=3) as xin_pool, \
                 tc.tile_pool(name="qkn", bufs=3) as qkn_pool, \
                 tc.tile_pool(name="proj_ps", bufs=4, space="PSUM") as proj_ps, \
                 tc.tile_pool(name="tp_ps", bufs=4, space="PSUM") as tp_ps, \
                 tc.tile_pool(name="ropet", bufs=3) as ropet:

                wT = projin.tile([128, 4, 3 * C], F32R)
                ones_sb = projin.tile([1, TOK], F32R)
                brow = projin.tile([1, 3 * C], F32R)
                peA = [projin.tile([128, NTB, D], F32, tag=f"peA{i}", name=f"peA{i}")
                       for i in range(2)]
                peB = [projin.tile([128, NTB, D], F32, tag=f"peB{i}", name=f"peB{i}")
                       for i in range(2)]

                nc.sync.dma_start(wT[:], wT_d.rearrange("(ko p) f -> p ko f", p=128))
                nc.sync.dma_start(ones_sb[:], ones_d[:])
                nc.sync.dma_start(brow[:], brow_d[:])
                for i, d in enumerate([peAq_d, peAk_d]):
                    nc.sync.dma_start(peA[i][:], d.rearrange("(tb p) d -> p tb d", p=128))
                for i, d in enumerate([peBq_d, peBk_d]):
                    nc.sync.dma_start(peB[i][:], d.rearrange("(tb p) d -> p tb d", p=128))

                for tt in range(NTT):
                    b, ttb = tt // NTB, tt % NTB
                    xt = xin_pool.tile([128, 4, 128], F32R, tag="xt")
                    nc.sync.dma_start(
                        xt[:],
                        xT_d[:, tt * 128:(tt + 1) * 128].rearrange(
                            "(ko p) t -> p ko t", p=128))
                    for fc in range(3):
                        ps = proj_ps.tile([128, 512], F32, tag="proj")
                        for kk in range(4):
                            nc.tensor.matmul(
                                ps[:], xt[:, kk, :],
                                wT[:, kk, fc * 512:(fc + 1) * 512],
                                start=(kk == 0), stop=False)
                        nc.tensor.matmul(
                            ps[:],
                            ones_sb[:, tt * 128:(tt + 1) * 128],
                            brow[:, fc * 512:(fc + 1) * 512],
                            start=False, stop=True)

                        if fc < 2:  # Q or K: rope -> fp32, then transpose
                            A = peA[fc][:, ttb, :]
                            Bp = peB[fc][:, ttb, :]
                            ps3 = ps[:].rearrange("p (h d) -> p h d", h=NH)
                            qk = qkn_pool.tile([128, C], F32R, tag=f"qk{fc}",
                                               name=f"qk{fc}")
                            t1 = ropet.tile([128, NH, D], F32, tag="t1")
                            nc.vector.tensor_tensor(
                                t1[:], ps3,
                                A[:, None, :].to_broadcast([128, NH, D]),
                                mybir.AluOpType.mult)
                            t2 = ropet.tile([128, NH, D], F32, tag="t2")
                            ps4 = ps[:].rearrange(
                                "p (h x two) -> p h x two", h=NH, two=2)
                            ps4_sw = ps4[:, :, :, ::-1]
                            nc.vector.tensor_tensor(
                                t2[:].rearrange("p h (x two) -> p h x two", two=2),
                                ps4_sw,
                                Bp[:, None, :].rearrange(
                                    "p o (x two) -> p o x two", two=2
                                ).to_broadcast([128, NH, D // 2, 2]),
                                mybir.AluOpType.mult)
                            nc.vector.tensor_tensor(
                                qk[:].rearrange("p (h d) -> p h d", h=NH),
                                t1[:], t2[:], mybir.AluOpType.add)
                            # transpose all 4 head-pairs of this token tile
                            tp = tp_ps.tile([128, 512], F32R, tag="tp")
                            for hp in range(NHP):
                                nc.tensor.matmul(
                                    tp[:, hp * 128:(hp + 1) * 128],
                                    qk[:, hp * 128:(hp + 1) * 128],
                                    id_f32[:],
                                    is_transpose=True)
                            dstt = (QT if fc == 0 else KT)[b]
                            nc.vector.tensor_copy(
                                dstt[:, :, ttb * 128:(ttb + 1) * 128],
                                tp[:].rearrange("p (hp t) -> p hp t", hp=NHP))
                        else:  # V: copy into V_ext layout (skip ones cols)
                            nc.vector.tensor_copy(
                                V_sb[:, tt, :528].rearrange(
                                    "p (h e) -> p h e", h=NH)[:, :, :D],
                                ps[:].rearrange("p (h d) -> p h d", h=NH))

            # ---------- phase 3: attention ----------
            persist2 = ctx.enter_context(tc.tile_pool(name="persist2", bufs=1))
            OT = [persist2.tile([96, NH, T], F32R, tag=f"OT{b}", name=f"OT{b}")
                  for b in range(BPC)]
            mT = [persist2.tile([128, NTB, T], BF16, tag=f"mT{b}", name=f"mT{b}")
                  for b in range(BPC)]
            for b in range(BPC):
                nc.sync.dma_start(
                    mT[b][:], nmT_d[b].rearrange("(kt p) q -> p kt q", p=128))

            with tc.tile_pool(name="s_ps", bufs=2, space="PSUM") as s_ps, \
                 tc.tile_pool(name="o_ps", bufs=2, space="PSUM") as o_ps, \
                 tc.tile_pool(name="alpha", bufs=3) as alpha_pool:

                for b in range(BPC):
                    for hp in range(NHP):
                        hA, hB = 2 * hp, 2 * hp + 1
                        for qc in range(NQC):
                            oA = o_ps.tile([96, QC], F32, tag="oA")
                            oB = o_ps.tile([96, QC], F32, tag="oB")
                            for kt in range(NTB):
                                sp = s_ps.tile([128, 2 * QC], F32, tag="s")
                                # S^T halves: head A cols 0:512, head B 512:1024
                                nc.tensor.matmul(
                                    sp[:, 0:QC],
                                    KT[b][0:64, hp, kt * 128:(kt + 1) * 128],
                                    QT[b][0:64, hp, qc * QC:(qc + 1) * QC],
                                    start=True, stop=True)
                                nc.tensor.matmul(
                                    sp[:, QC:2 * QC],
                                    KT[b][64:128, hp, kt * 128:(kt + 1) * 128],
                                    QT[b][64:128, hp, qc * QC:(qc + 1) * QC],
                                    start=True, stop=True)
                                al = alpha_pool.tile([128, 2 * QC], BF16, tag="al")
                                nc.scalar.activation(
                                    al[:], sp[:],
                                    mybir.ActivationFunctionType.Exp,
                                    scale=1.0 / TP)
                                nc.vector.tensor_tensor(
                                    al[:].rearrange("p (h q) -> p h q", h=2),
                                    al[:].rearrange("p (h q) -> p h q", h=2),
                                    mT[b][:, kt, qc * QC:(qc + 1) * QC][:, None, :]
                                    .to_broadcast([128, 2, QC]),
                                    mybir.AluOpType.mult)
                                vbase = b * NTB + kt
                                nc.tensor.matmul(
                                    oA[:],
                                    V_sb[:, vbase, hA * 66:hA * 66 + 96],
                                    al[:, 0:QC],
                                    start=(kt == 0), stop=(kt == NTB - 1))
                                nc.tensor.matmul(
                                    oB[:],
                                    V_sb[:, vbase, hB * 66:hB * 66 + 96],
                                    al[:, QC:2 * QC],
                                    start=(kt == 0), stop=(kt == NTB - 1))
                            nc.vector.tensor_copy(
                                OT[b][:, hA, qc * QC:(qc + 1) * QC], oA[:])
                            nc.vector.tensor_copy(
                                OT[b][:, hB, qc * QC:(qc + 1) * QC], oB[:])

            # ---------- phase 4: final transpose + normalize + store ----------
            with tc.tile_pool(name="fin_ps", bufs=4, space="PSUM") as fin_ps, \
                 tc.tile_pool(name="fin_sb", bufs=3) as fin_sb:
                for b in range(BPC):
                    for qt in range(NTB):
                        fins = []
                        for half in range(2):
                            fp = fin_ps.tile([128, 4 * 96], F32R, tag=f"fin{half}",
                                             name=f"fin{half}")
                            for hh in range(4):
                                h = half * 4 + hh
                                nc.tensor.matmul(
                                    fp[:, hh * 96:(hh + 1) * 96],
                                    OT[b][:, h, qt * 128:(qt + 1) * 128],
                                    id_f32[0:96, 0:96],
                                    is_transpose=True)
                            fins.append(fp)
                        out_sb = fin_sb.tile([128, C], F32, tag="out")
                        for half in range(2):
                            fp = fins[half]
                            rc = fin_sb.tile([128, 4], F32, tag=f"rc{half}",
                                             name=f"rc{half}")
                            nc.vector.reciprocal(rc[:], fp[:, 64::96])
                            nc.vector.tensor_tensor(
                                out_sb[:, half * 256:(half + 1) * 256].rearrange(
                                    "p (h d) -> p h d", h=4),
                                fp[:].rearrange("p (h e) -> p h e", e=96)[:, :, :D],
                                rc[:][:, :, None].to_broadcast([128, 4, D]),
                                mybir.AluOpType.mult)
                        nc.sync.dma_start(
                            y_d[(b * NTB + qt) * 128:(b * NTB + qt + 1) * 128, :],
                            out_sb[:])

    nc.compile()
    return nc


_NC_CACHE = None


def _get_nc():
    global _NC_CACHE
    if _NC_CACHE is None:
        _NC_CACHE = build_nc()
    return _NC_CACHE


def prep_inputs(x, pe0, pe1, pe2, mask, W_qkv, b_qkv):
    """Host-side layout prep + per-core sharding. Returns list of in_maps."""
    x = np.asarray(x, dtype=np.float32)
    pe0 = np.asarray(pe0, dtype=np.float32).reshape(T, D)
    pe1 = np.asarray(pe1, dtype=np.float32).reshape(T, D)
    pe2 = np.asarray(pe2, dtype=np.float32).reshape(T, D)
    mask = np.asarray(mask).reshape(B, T, T)
    W_qkv = np.asarray(W_qkv, dtype=np.float32)
    b_qkv = np.asarray(b_qkv, dtype=np.float32)

    wT = np.ascontiguousarray(W_qkv.T)                      # [C, 3C]
    brow = np.ascontiguousarray(b_qkv[None, :])             # [1, 3C]
    ones_row = np.ones((1, TOK), dtype=np.float32)

    # rope tables: q' = q*A + swap(q)*B' ; A=pe0*pe2, B=pe1*pe2 (sign-folded)
    Aq = pe0 * pe2
    Bq = pe1 * pe2
    Ak = pe0 / pe2
    Bk = pe1 / pe2
    sign = np.ones((T, D), dtype=np.float32)
    sign[:, 0::2] = -1.0
    peBq = np.ascontiguousarray(Bq * sign)
    peBk = np.ascontiguousarray(Bk * sign)

    notmask = (~mask).astype(ml_dtypes.bfloat16)            # [B,T,T] {0,1}
    in_maps = []
    for c in range(N_CORES):
        bs = slice(c * BPC, (c + 1) * BPC)
        xc = np.ascontiguousarray(
            x[bs].reshape(TOK, C).T)                         # [C, TOK]
        nmT = np.ascontiguousarray(
            notmask[bs].transpose(0, 2, 1))                  # [BPC, T(kv), T(q)]
        in_maps.append(dict(
            xT=xc, wT=wT, brow=brow, ones_row=ones_row,
            peAq=np.ascontiguousarray(Aq), peBq=peBq,
            peAk=np.ascontiguousarray(Ak), peBk=peBk,
            nmT=nmT,
        ))
    return in_maps


def assemble_output(results):
    out = np.empty((B, T, C), dtype=np.float32)
    for c in range(N_CORES):
        out[c * BPC:(c + 1) * BPC] = results[c]["y"].reshape(BPC, T, C)
    return out


def kernel(x, pe0, pe1, pe2, mask, W_qkv, b_qkv):
    nc = _get_nc()
    in_maps = prep_inputs(x, pe0, pe1, pe2, mask, W_qkv, b_qkv)
    res = run_bass_kernel_spmd(nc, in_maps, core_ids=list(range(N_CORES)))
    return assemble_output(res.results)
